# revision 1
# baseline (speedup 1.0000x reference)
"""Self-contained Trainium2 (Bass/Tile) kernel for the nn_Encoder problem.

kernel(**inputs) takes the FULL unsharded inputs (as produced by
setup_inputs()) and returns the FULL [4, 2048, 1024] fp32 output.

Internally: 8-way data-parallel over tokens (2 NeuronCores per batch row,
1024 query-tokens each; K/V computed redundantly per pair => no collectives).
Matmuls in fp32r; attention P/V and FFN G/W2 in bf16.
"""
import os
import numpy as np

import concourse.bass as bass
import concourse.bacc as bacc
import concourse.mybir as mybir
import concourse.tile as tile

F32 = mybir.dt.float32
F32R = mybir.dt.float32r
BF16 = mybir.dt.bfloat16
AF = mybir.ActivationFunctionType
ALU = mybir.AluOpType

E = 1024
FF = 4096
B, S = 4, 2048
T = 1024      # own tokens per core
R = 2048      # row tokens (for K/V)
P = 128
NE = E // P   # 8
NFF = FF // P # 32
NT = T // P   # 8
NR = R // P   # 16
EPS = 1e-5


def build(nc):
    # ---- DRAM I/O ----
    xrT = nc.dram_tensor("xrT", [E, R], F32R, kind="ExternalInput")   # row, feature-major
    xo = nc.dram_tensor("xo", [T, E], F32, kind="ExternalInput")      # own block, token-major
    wqT = nc.dram_tensor("wqT", [E, E], F32R, kind="ExternalInput")   # (Wq g1)^T / 32
    wkT = nc.dram_tensor("wkT", [E, E], F32R, kind="ExternalInput")
    wvT = nc.dram_tensor("wvT", [E, E], F32R, kind="ExternalInput")
    woT = nc.dram_tensor("woT", [E, E], F32R, kind="ExternalInput")
    w1T = nc.dram_tensor("w1T", [E, FF], F32R, kind="ExternalInput")
    w2T = nc.dram_tensor("w2T", [FF, E], BF16, kind="ExternalInput")
    bq = nc.dram_tensor("bq", [E], F32, kind="ExternalInput")
    bk = nc.dram_tensor("bk", [E], F32, kind="ExternalInput")
    bv = nc.dram_tensor("bv", [E], F32, kind="ExternalInput")
    bo = nc.dram_tensor("bo", [E], F32, kind="ExternalInput")
    nwqs = nc.dram_tensor("nwqs", [E], F32, kind="ExternalInput")   # -colsum(wqT)
    nwks = nc.dram_tensor("nwks", [E], F32, kind="ExternalInput")
    nwvs = nc.dram_tensor("nwvs", [E], F32, kind="ExternalInput")
    b1 = nc.dram_tensor("b1", [FF], F32, kind="ExternalInput")
    b2 = nc.dram_tensor("b2", [E], F32, kind="ExternalInput")
    g3 = nc.dram_tensor("g3", [E], F32, kind="ExternalInput")
    b3 = nc.dram_tensor("b3", [E], F32, kind="ExternalInput")
    ident = nc.dram_tensor("ident", [P, P], F32R, kind="ExternalInput")
    ones_in = nc.dram_tensor("ones_in", [P, 1], F32R, kind="ExternalInput")
    y = nc.dram_tensor("y", [T, E], F32, kind="ExternalOutput")

    def bcast_ap(vec_t, n):
        a = vec_t.ap()
        return bass.AP(tensor=a.tensor, offset=a.offset, ap=[[0, P], [1, n]])

    with tile.TileContext(nc) as tc:
        consts_cm = tc.tile_pool(name="consts", bufs=1)
        consts = consts_cm.__enter__()
        dram_cm = tc.tile_pool(name="dram", bufs=1, space="DRAM")
        dram = dram_cm.__enter__()

        ident_sb = consts.tile([P, P], F32R, tag="ident")
        nc.sync.dma_start(out=ident_sb, in_=ident.ap())
        ones_sb = consts.tile([P, 1], F32R, tag="ones")
        nc.sync.dma_start(out=ones_sb, in_=ones_in.ap())
        eps_row = consts.tile([1, 1], F32, tag="eps_row")
        nc.vector.memset(eps_row, EPS)
        eps_col = consts.tile([P, 1], F32, tag="eps_col")
        nc.vector.memset(eps_col, EPS)
        bq_sb = consts.tile([P, NE], F32, tag="bq")
        nc.sync.dma_start(out=bq_sb, in_=bq.ap().rearrange("(t p) -> p t", p=P))
        bk_sb = consts.tile([P, NE], F32, tag="bk")
        nc.sync.dma_start(out=bk_sb, in_=bk.ap().rearrange("(t p) -> p t", p=P))
        nwqs_sb = consts.tile([P, NE], F32, tag="nwqs")
        nc.sync.dma_start(out=nwqs_sb, in_=nwqs.ap().rearrange("(t p) -> p t", p=P))
        nwks_sb = consts.tile([P, NE], F32, tag="nwks")
        nc.sync.dma_start(out=nwks_sb, in_=nwks.ap().rearrange("(t p) -> p t", p=P))
        b1_sb = consts.tile([P, NFF], F32, tag="b1")
        nc.sync.dma_start(out=b1_sb, in_=b1.ap().rearrange("(t p) -> p t", p=P))
        bv_b = consts.tile([P, E], F32, tag="bv_b")
        nc.sync.dma_start(out=bv_b, in_=bcast_ap(bv, E))
        nwvs_b = consts.tile([P, E], F32, tag="nwvs_b")
        nc.sync.dma_start(out=nwvs_b, in_=bcast_ap(nwvs, E))
        bo_b = consts.tile([P, E], F32, tag="bo_b")
        nc.sync.dma_start(out=bo_b, in_=bcast_ap(bo, E))
        b2_b = consts.tile([P, E], F32, tag="b2_b")
        nc.sync.dma_start(out=b2_b, in_=bcast_ap(b2, E))
        g3_b = consts.tile([P, E], F32, tag="g3_b")
        nc.sync.dma_start(out=g3_b, in_=bcast_ap(g3, E))
        b3_b = consts.tile([P, E], F32, tag="b3_b")
        nc.sync.dma_start(out=b3_b, in_=bcast_ap(b3, E))

        q_d = dram.tile([E, T], F32R, tag="q_d")
        k_d = dram.tile([E, R], F32R, tag="k_d")
        v_d = dram.tile([R, E], BF16, tag="v_d")
        pt_d = dram.tile([R, T], BF16, tag="pt_d")
        h_d = dram.tile([T, E], F32, tag="h_d")
        stat_d = dram.tile([3, R], F32, tag="stat_d")   # mean, rstd, mean*rstd rows

        def sA_qkv():
            """LN1 stats + Q/K/V directly from x with LayerNorm output-fixup."""
            with tc.tile_pool(name="sAx", bufs=1) as xpool, \
                 tc.tile_pool(name="sAkeep", bufs=1) as keepp, \
                 tc.tile_pool(name="sAbc", bufs=1) as bcp:
                xt = []
                for k in range(NE):
                    x_k = xpool.tile([P, R], F32R, tag=f"x{k}", name=f"x{k}")
                    for c in range(4):
                        cs = slice(c * 512, (c + 1) * 512)
                        nc.sync.dma_start(out=x_k[:, cs], in_=xrT.ap()[k * P:(k + 1) * P, cs])
                    xt.append(x_k)
                rstd = keepp.tile([1, R], F32, tag="rstd")
                mr = keepp.tile([1, R], F32, tag="mr")
                # --- stats scope (freed before the weight pools open) ---
                with tc.tile_pool(name="sAsq", bufs=3) as sqp, \
                     tc.tile_pool(name="sArow", bufs=1) as rowp, \
                     tc.tile_pool(name="sArps", bufs=2, space="PSUM") as rpsA:
                    srow = rowp.tile([1, R], F32, tag="srow")
                    qrow = rowp.tile([1, R], F32, tag="qrow")
                    mean = rowp.tile([1, R], F32, tag="mean")
                    for c in range(R // 512):
                        cs = slice(c * 512, (c + 1) * 512)
                        ps_s = rpsA.tile([1, 512], F32, tag="ps_s")
                        ps_q = rpsA.tile([1, 512], F32, tag="ps_q")
                        for k in range(NE):
                            sq = sqp.tile([P, 512], F32R, tag="sq")
                            nc.vector.tensor_mul(sq, xt[k][:, cs], xt[k][:, cs])
                            nc.tensor.matmul(ps_s[:], ones_sb[:], xt[k][:, cs],
                                             start=(k == 0), stop=(k == NE - 1))
                            nc.tensor.matmul(ps_q[:], ones_sb[:], sq[:],
                                             start=(k == 0), stop=(k == NE - 1))
                        nc.vector.tensor_copy(out=srow[:, cs], in_=ps_s[:])
                        nc.vector.tensor_copy(out=qrow[:, cs], in_=ps_q[:])
                    nc.vector.tensor_scalar_mul(mean, srow[:], 1.0 / E)
                    msq = rowp.tile([1, R], F32, tag="tmp", bufs=2, name="msq")
                    nc.vector.tensor_mul(msq, mean[:], mean[:])
                    var = rowp.tile([1, R], F32, tag="tmp", bufs=2, name="var")
                    nc.vector.scalar_tensor_tensor(out=var, in0=qrow[:], scalar=1.0 / E,
                                                   in1=msq[:], op0=ALU.mult, op1=ALU.subtract)
                    sd = rowp.tile([1, R], F32, tag="tmp", bufs=2, name="sd")
                    nc.scalar.activation(out=sd, in_=var[:], func=AF.Sqrt, bias=eps_row[:],
                                         scale=1.0)
                    nc.vector.reciprocal(rstd, sd[:])
                    nc.vector.tensor_mul(mr, mean[:], rstd[:])
                    nc.sync.dma_start(out=stat_d[0:1, :], in_=mean[:])
                    nc.sync.dma_start(out=stat_d[1:2, :], in_=rstd[:])
                r_b = bcp.tile([P, R], F32, tag="r_b")
                nc.gpsimd.partition_broadcast(r_b, rstd[:])
                mr_b = bcp.tile([P, R], F32, tag="mr_b")
                nc.gpsimd.partition_broadcast(mr_b, mr[:])
                m_v = bcp.tile([P, NR], F32, tag="m_v")
                nc.sync.dma_start(out=m_v, in_=stat_d[0:1, :].rearrange("a (t p) -> (a p) t", p=P))
                r_v = bcp.tile([P, NR], F32, tag="r_v")
                nc.sync.dma_start(out=r_v, in_=stat_d[1:2, :].rearrange("a (t p) -> (a p) t", p=P))
                wvwork_cm = tc.tile_pool(name="sAwv", bufs=1)
                wvp = wvwork_cm.__enter__()
                wsp_cm = tc.tile_pool(name="sAw", bufs=3)
                wsp = wsp_cm.__enter__()
                outp_cm = tc.tile_pool(name="sAout", bufs=2)
                outp = outp_cm.__enter__()
                fxp_cm = tc.tile_pool(name="sAfix", bufs=2)
                fxp = fxp_cm.__enter__()
                psA_cm = tc.tile_pool(name="sAps", bufs=2, space="PSUM")
                psA = psA_cm.__enter__()

                # --- V0 = x^T-stationary @ wvT, fixup to token-major V (bf16) ---
                wv_t = {}
                for k in range(NE):
                    for c in range(2):
                        w = wvp.tile([P, 512], F32R, tag=f"wv{k}_{c}", name=f"wv{k}_{c}")
                        nc.sync.dma_start(out=w, in_=wvT.ap()[k * P:(k + 1) * P,
                                                              c * 512:(c + 1) * 512])
                        wv_t[(k, c)] = w
                for rm in range(NR):
                    psv = psA.tile([P, E], F32, tag="psv")
                    for k in range(NE):
                        for c in range(2):
                            nc.tensor.matmul(psv[:, c * 512:(c + 1) * 512],
                                             xt[k][:, rm * P:(rm + 1) * P], wv_t[(k, c)][:],
                                             start=(k == 0), stop=(k == NE - 1))
                    w0 = fxp.tile([P, E], F32, tag="vfx", name="vfx0")
                    nc.vector.scalar_tensor_tensor(out=w0, in0=nwvs_b[:], scalar=m_v[:, rm:rm + 1],
                                                   in1=psv[:], op0=ALU.mult, op1=ALU.add)
                    t1 = fxp.tile([P, E], F32, tag="vfx", name="vfx1")
                    nc.vector.tensor_scalar_mul(t1, w0[:], r_v[:, rm:rm + 1])
                    v_sb = outp.tile([P, E], BF16, tag="v_sb")
                    nc.gpsimd.tensor_add(v_sb, t1[:], bv_b[:])
                    nc.sync.dma_start(out=v_d[rm * P:(rm + 1) * P, :], in_=v_sb[:])

                # --- Q0/K0 weight-stationary, fixup feature-major ---
                def qk_block(wT_d, out_d, ncols, nws_sb, bias_sb):
                    nh = ncols // T   # 1 for Q, 2 for K
                    for mg in range(2):
                        w_g = []
                        for k in range(NE):
                            w = wsp.tile([P, 512], F32R, tag="w_s", bufs=10, name="w_s")
                            nc.sync.dma_start(out=w, in_=wT_d.ap()[k * P:(k + 1) * P,
                                                                   mg * 512:(mg + 1) * 512])
                            w_g.append(w)
                        for mi in range(4):
                            m = mg * 4 + mi
                            for half in range(nh):
                                hs = slice(half * T, (half + 1) * T)
                                psq = psA.tile([P, T], F32, tag="psqk", name="psqk")
                                for k in range(NE):
                                    for c in range(2):
                                        src = slice(half * T + c * 512, half * T + (c + 1) * 512)
                                        nc.tensor.matmul(psq[:, c * 512:(c + 1) * 512],
                                                         w_g[k][:, mi * P:(mi + 1) * P],
                                                         xt[k][:, src],
                                                         start=(k == 0), stop=(k == NE - 1))
                                t0 = fxp.tile([P, T], F32, tag="qkfx", name="qkfx0")
                                nc.vector.scalar_tensor_tensor(out=t0, in0=psq[:], scalar=1.0,
                                                               in1=r_b[:, hs], op0=ALU.mult,
                                                               op1=ALU.mult)
                                u = fxp.tile([P, T], F32, tag="qkfx", name="qkfx1")
                                nc.vector.tensor_scalar(out=u, in0=mr_b[:, hs],
                                                        scalar1=nws_sb[:, m:m + 1],
                                                        scalar2=bias_sb[:, m:m + 1],
                                                        op0=ALU.mult, op1=ALU.add)
                                o_sb = outp.tile([P, T], F32R, tag="qk_sb", name="qk_sb")
                                nc.gpsimd.tensor_add(o_sb, t0[:], u[:])
                                nc.sync.dma_start(
                                    out=out_d[m * P:(m + 1) * P, half * T:(half + 1) * T],
                                    in_=o_sb[:])
                qk_block(wqT, q_d, T, nwqs_sb, bq_sb)
                qk_block(wkT, k_d, R, nwks_sb, bk_sb)
                psA_cm.__exit__(None, None, None)
                fxp_cm.__exit__(None, None, None)
                outp_cm.__exit__(None, None, None)
                wsp_cm.__exit__(None, None, None)
                wvwork_cm.__exit__(None, None, None)

        def sB_scores():
            """S = Q^T K, exp (|S|<2 by construction, no max pass), P^T -> bf16."""
            with tc.tile_pool(name="sBq", bufs=1) as qp, \
                 tc.tile_pool(name="sBk", bufs=1) as kp, \
                 tc.tile_pool(name="sBw", bufs=1) as wkb, \
                 tc.tile_pool(name="sBsm", bufs=2) as smp, \
                 tc.tile_pool(name="sBps", bufs=1, space="PSUM") as psB, \
                 tc.tile_pool(name="sBtp", bufs=4, space="PSUM") as psBt:
                q_sb = []
                k_sb = []
                for m in range(NE):
                    qt = qp.tile([P, T], F32R, tag=f"q{m}", name=f"q{m}")
                    nc.sync.dma_start(out=qt, in_=q_d[m * P:(m + 1) * P, :])
                    q_sb.append(qt)
                    kt_ = kp.tile([P, R], F32R, tag=f"k{m}", name=f"k{m}")
                    nc.sync.dma_start(out=kt_, in_=k_d[m * P:(m + 1) * P, :])
                    k_sb.append(kt_)
                p_tiles = []
                for qm in range(NT):
                    qs = slice(qm * P, (qm + 1) * P)
                    p_tile = wkb.tile([P, R], F32R, tag=f"p{qm}", name=f"p{qm}")
                    p_tiles.append(p_tile)
                    acc = smp.tile([P, 4], F32, tag="acc")
                    ps_s = [psB.tile([P, 512], F32, tag=f"ps_s{c}", name=f"ps_s{c}")
                            for c in range(4)]
                    for k in range(NE):
                        for c in range(4):
                            nc.tensor.matmul(ps_s[c][:], q_sb[k][:, qs],
                                             k_sb[k][:, c * 512:(c + 1) * 512],
                                             start=(k == 0), stop=(k == NE - 1))
                    for c in range(4):
                        nc.scalar.activation(out=p_tile[:, c * 512:(c + 1) * 512],
                                             in_=ps_s[c][:],
                                             func=AF.Exp, accum_out=acc[:, c:c + 1])
                    s01 = smp.tile([P, 1], F32, tag="s01")
                    nc.vector.tensor_add(s01, acc[:, 0:1], acc[:, 1:2])
                    s23 = smp.tile([P, 1], F32, tag="s23")
                    nc.vector.tensor_add(s23, acc[:, 2:3], acc[:, 3:4])
                    rsum = smp.tile([P, 1], F32, tag="rsum")
                    nc.vector.tensor_add(rsum, s01[:], s23[:])
                    recip = smp.tile([P, 1], F32, tag="recip")
                    nc.vector.reciprocal(recip, rsum[:])
                    nc.vector.tensor_scalar_mul(p_tile, p_tile[:], recip[:])
                    for kt in range(NR):
                        tp = psBt.tile([P, P], F32R, tag="tp")
                        nc.tensor.transpose(tp, p_tiles[qm][:, kt * P:(kt + 1) * P], ident_sb[:])
                        ptc = smp.tile([P, P], BF16, tag="ptc", bufs=4, name="ptc")
                        nc.scalar.copy(out=ptc, in_=tp[:])
                        nc.sync.dma_start(out=pt_d[kt * P:(kt + 1) * P, qm * P:(qm + 1) * P],
                                          in_=ptc[:])

        def sC_attnout():
            """AOT = V^T P^T (bf16 in, fp32 psum), then O = AO^T WoT + bo, h = xo + O."""
            with tc.tile_pool(name="sCv", bufs=1) as vbp, \
                 tc.tile_pool(name="sCpt", bufs=1) as ptp, \
                 tc.tile_pool(name="sCao", bufs=1) as aop, \
                 tc.tile_pool(name="sCwo", bufs=1) as wop, \
                 tc.tile_pool(name="sCw", bufs=3) as wkc, \
                 tc.tile_pool(name="sCps", bufs=3, space="PSUM") as psC:
                v_back = []
                pt_sb = []
                for kt in range(NR):
                    vb = vbp.tile([P, E], BF16, tag=f"vb{kt}", name=f"vb{kt}")
                    nc.sync.dma_start(out=vb, in_=v_d[kt * P:(kt + 1) * P, :])
                    v_back.append(vb)
                    pb = ptp.tile([P, T], BF16, tag=f"pt{kt}", name=f"pt{kt}")
                    nc.sync.dma_start(out=pb, in_=pt_d[kt * P:(kt + 1) * P, :])
                    pt_sb.append(pb)
                wo_t = {}
                for k in range(NE):
                    for c in range(2):
                        w = wop.tile([P, 512], F32R, tag=f"wo{k}_{c}", name=f"wo{k}_{c}")
                        nc.sync.dma_start(out=w, in_=woT.ap()[k * P:(k + 1) * P,
                                                             c * 512:(c + 1) * 512])
                        wo_t[(k, c)] = w
                ao_sb = []
                for m in range(NE):
                    psa = psC.tile([P, T], F32, tag="psa", bufs=2)
                    for kt in range(NR):
                        for c in range(T // 512):
                            cs = slice(c * 512, (c + 1) * 512)
                            nc.tensor.matmul(psa[:, cs], v_back[kt][:, m * P:(m + 1) * P],
                                             pt_sb[kt][:, cs],
                                             start=(kt == 0), stop=(kt == NR - 1))
                    ao_m = aop.tile([P, T], F32R, tag=f"ao{m}", name=f"ao{m}")
                    nc.scalar.copy(out=ao_m, in_=psa[:])
                    ao_sb.append(ao_m)
                for tm in range(NT):
                    pso = psC.tile([P, E], F32, tag="pso", bufs=2)
                    for k in range(NE):
                        for c in range(2):
                            nc.tensor.matmul(pso[:, c * 512:(c + 1) * 512],
                                             ao_sb[k][:, tm * P:(tm + 1) * P], wo_t[(k, c)][:],
                                             start=(k == 0), stop=(k == NE - 1))
                    xo_t = wkc.tile([P, E], F32, tag="xo_t")
                    nc.sync.dma_start(out=xo_t, in_=xo.ap()[tm * P:(tm + 1) * P, :])
                    t0 = wkc.tile([P, E], F32, tag="t0")
                    nc.vector.tensor_add(t0, pso[:], bo_b[:])
                    h_t = wkc.tile([P, E], F32, tag="h_t")
                    nc.vector.tensor_add(h_t, t0[:], xo_t[:])
                    nc.sync.dma_start(out=h_d[tm * P:(tm + 1) * P, :], in_=h_t[:])

        def sD_ffn():
            """LN2 + transpose, F1 (G bf16, resident), F2 + residual + LN3."""
            with tc.tile_pool(name="sDhn", bufs=1) as hnp, \
                 tc.tile_pool(name="sDg", bufs=1) as gp, \
                 tc.tile_pool(name="sDw", bufs=3) as wkd, \
                 tc.tile_pool(name="sDt", bufs=3) as t6, \
                 tc.tile_pool(name="sDst", bufs=2) as st6:
                hnT = [hnp.tile([P, T], F32R, tag=f"hnT{k}", name=f"hnT{k}")
                       for k in range(NE)]
                psDt_cm = tc.tile_pool(name="sDtp", bufs=2, space="PSUM")
                psDt = psDt_cm.__enter__()
                for tm in range(NT):
                    h_t = t6.tile([P, E], F32, tag="h_in", bufs=2, name="h_in")
                    nc.sync.dma_start(out=h_t, in_=h_d[tm * P:(tm + 1) * P, :])
                    stats = st6.tile([P, 2, 6], F32, tag="stats")
                    hg = h_t[:].rearrange("p (g d) -> p g d", g=2)
                    for g in range(2):
                        nc.vector.bn_stats(out=stats[:, g, :], in_=hg[:, g, :])
                    mv = st6.tile([P, 2], F32, tag="mv")
                    nc.vector.bn_aggr(out=mv, in_=stats[:])
                    sd = st6.tile([P, 1], F32, tag="sd")
                    nc.scalar.activation(out=sd, in_=mv[:, 1:2], func=AF.Sqrt,
                                         bias=eps_col[:], scale=1.0)
                    rinv = st6.tile([P, 1], F32, tag="rinv")
                    nc.vector.reciprocal(rinv, sd[:])
                    hn = t6.tile([P, E], F32R, tag="hn", bufs=2, name="hn")
                    nc.vector.tensor_scalar(out=hn, in0=h_t[:], scalar1=mv[:, 0:1],
                                            scalar2=rinv[:], op0=ALU.subtract, op1=ALU.mult)
                    for et in range(NE):
                        tp = psDt.tile([P, P], F32R, tag="tp5")
                        nc.tensor.transpose(tp, hn[:, et * P:(et + 1) * P], ident_sb[:])
                        nc.scalar.copy(out=hnT[et][:, tm * P:(tm + 1) * P], in_=tp[:])
                psDt_cm.__exit__(None, None, None)
                # F1: G = relu(W1 hn + b1), bf16, full T resident
                g_t = []
                with tc.tile_pool(name="sDpsg", bufs=3, space="PSUM") as psg6:
                    for fg in range(NFF // 4):
                        w1_g = []
                        for k in range(NE):
                            w = wkd.tile([P, 512], F32R, tag="w1_s", bufs=10, name="w1_s")
                            nc.sync.dma_start(out=w, in_=w1T.ap()[k * P:(k + 1) * P,
                                                                  fg * 512:(fg + 1) * 512])
                            w1_g.append(w)
                        for fi in range(4):
                            fm = fg * 4 + fi
                            g = gp.tile([P, T], BF16, tag=f"g{fm}", name=f"g{fm}")
                            for c in range(2):
                                cs = slice(c * 512, (c + 1) * 512)
                                psg = psg6.tile([P, 512], F32, tag="psg", bufs=4)
                                for k in range(NE):
                                    nc.tensor.matmul(psg[:],
                                                     w1_g[k][:, fi * P:(fi + 1) * P],
                                                     hnT[k][:, cs],
                                                     start=(k == 0), stop=(k == NE - 1))
                                nc.scalar.activation(out=g[:, cs], in_=psg[:], func=AF.Relu,
                                                     bias=b1_sb[:, fm:fm + 1], scale=1.0)
                            g_t.append(g)
                # F2 + residual + LN3 per 512-token chunk
                for ch in range(2):
                    with tc.tile_pool(name=f"sDpsf{ch}", bufs=1, space="PSUM") as psf6:
                        psf = [psf6.tile([P, E], F32, tag=f"psf{tm}", name=f"psf{tm}")
                               for tm in range(4)]
                        for fm in range(NFF):
                            w2r = wkd.tile([P, E], BF16, tag="w2_s", bufs=6, name="w2_s")
                            nc.sync.dma_start(out=w2r, in_=w2T.ap()[fm * P:(fm + 1) * P, :])
                            for tm in range(4):
                                gtm = ch * 4 + tm
                                for c in range(2):
                                    nc.tensor.matmul(psf[tm][:, c * 512:(c + 1) * 512],
                                                     g_t[fm][:, gtm * P:(gtm + 1) * P],
                                                     w2r[:, c * 512:(c + 1) * 512],
                                                     start=(fm == 0), stop=(fm == NFF - 1))
                        for tm in range(4):
                            gtm = ch * 4 + tm
                            h_t = t6.tile([P, E], F32, tag="h_in", bufs=2, name="h_in2")
                            nc.sync.dma_start(out=h_t, in_=h_d[gtm * P:(gtm + 1) * P, :])
                            t1 = t6.tile([P, E], F32, tag="chain", name="t1")
                            nc.vector.tensor_add(t1, psf[tm][:], h_t[:])
                            op = t6.tile([P, E], F32, tag="chain", name="op")
                            nc.vector.tensor_add(op, t1[:], b2_b[:])
                            stats = st6.tile([P, 2, 6], F32, tag="stats7")
                            og = op[:].rearrange("p (g d) -> p g d", g=2)
                            for g in range(2):
                                nc.vector.bn_stats(out=stats[:, g, :], in_=og[:, g, :])
                            mv = st6.tile([P, 2], F32, tag="mv7")
                            nc.vector.bn_aggr(out=mv, in_=stats[:])
                            sd = st6.tile([P, 1], F32, tag="sd7")
                            nc.scalar.activation(out=sd, in_=mv[:, 1:2], func=AF.Sqrt,
                                                 bias=eps_col[:], scale=1.0)
                            rinv = st6.tile([P, 1], F32, tag="rinv7")
                            nc.vector.reciprocal(rinv, sd[:])
                            n = t6.tile([P, E], F32, tag="chain", name="n")
                            nc.vector.tensor_scalar(out=n, in0=op[:], scalar1=mv[:, 0:1],
                                                    scalar2=rinv[:], op0=ALU.subtract,
                                                    op1=ALU.mult)
                            yg = t6.tile([P, E], F32, tag="chain", name="yg")
                            nc.vector.tensor_mul(yg, n[:], g3_b[:])
                            yt = t6.tile([P, E], F32, tag="chain", name="yt")
                            nc.vector.tensor_add(yt, yg[:], b3_b[:])
                            nc.sync.dma_start(out=y.ap()[gtm * P:(gtm + 1) * P, :], in_=yt[:])

        stages = [sA_qkv, sB_scores, sC_attnout, sD_ffn]
        for _rep in range(int(os.environ.get("ENC_REPS", "1"))):
            for f in stages:
                f()

        consts_cm.__exit__(None, None, None)
        dram_cm.__exit__(None, None, None)


# ======================= host-side prep / assembly =========================

def prep_inputs(inputs):
    import ml_dtypes
    src = np.asarray(inputs["src_embs"], np.float32)   # [B, S, E]
    g1 = np.asarray(inputs["g1"], np.float32)
    b1ln = np.asarray(inputs["b1"], np.float32)
    g2 = np.asarray(inputs["g2"], np.float32)
    b2ln = np.asarray(inputs["b2"], np.float32)

    Wq, bq = np.asarray(inputs["Wq_w"], np.float32), np.asarray(inputs["Wq_b"], np.float32)
    Wk, bk = np.asarray(inputs["Wk_w"], np.float32), np.asarray(inputs["Wk_b"], np.float32)
    Wv, bv = np.asarray(inputs["Wv_w"], np.float32), np.asarray(inputs["Wv_b"], np.float32)
    Wo, bo = np.asarray(inputs["Wo_w"], np.float32), np.asarray(inputs["Wo_b"], np.float32)
    W1, b1f = np.asarray(inputs["W1_w"], np.float32), np.asarray(inputs["W1_b"], np.float32)
    W2, b2f = np.asarray(inputs["W2_w"], np.float32), np.asarray(inputs["W2_b"], np.float32)

    scale = 1.0 / np.sqrt(np.float32(E))
    wqT = ((Wq * g1[None, :]).T * scale).astype(np.float32)
    bq_eff = ((bq + Wq @ b1ln) * scale).astype(np.float32)
    wkT = (Wk * g1[None, :]).T.astype(np.float32)
    bk_eff = (bk + Wk @ b1ln).astype(np.float32)
    wvT = (Wv * g1[None, :]).T.astype(np.float32)
    bv_eff = (bv + Wv @ b1ln).astype(np.float32)
    woT = Wo.T.astype(np.float32)
    w1T = (W1 * g2[None, :]).T.astype(np.float32)
    b1_eff = (b1f + W1 @ b2ln).astype(np.float32)
    w2T = W2.T.astype(ml_dtypes.bfloat16)

    shared = dict(
        wqT=np.ascontiguousarray(wqT), wkT=np.ascontiguousarray(wkT),
        wvT=np.ascontiguousarray(wvT), woT=np.ascontiguousarray(woT),
        w1T=np.ascontiguousarray(w1T), w2T=np.ascontiguousarray(w2T),
        bq=bq_eff, bk=bk_eff, bv=bv_eff, bo=bo,
        nwqs=(-wqT.sum(axis=0)).astype(np.float32),
        nwks=(-wkT.sum(axis=0)).astype(np.float32),
        nwvs=(-wvT.sum(axis=0)).astype(np.float32),
        b1=b1_eff, b2=b2f,
        g3=np.asarray(inputs["g3"], np.float32), b3=np.asarray(inputs["b3"], np.float32),
        ident=np.eye(P, dtype=np.float32),
        ones_in=np.ones((P, 1), np.float32),
    )
    in_maps = []
    for c in range(8):
        b, half = c // 2, c % 2
        row = src[b]
        own = row[half * T:(half + 1) * T]
        other = row[(1 - half) * T:(2 - half) * T]
        xr = np.concatenate([own, other], axis=0)
        m = dict(shared)
        m["xrT"] = np.ascontiguousarray(xr.T)
        m["xo"] = np.ascontiguousarray(own)
        in_maps.append(m)
    return in_maps


def assemble_output(results):
    out = np.zeros((B, S, E), np.float32)
    for c in range(8):
        b, half = c // 2, c % 2
        out[b, half * T:(half + 1) * T] = results[c]["y"]
    return out


def build_nc():
    nc = bacc.Bacc("TRN2", target_bir_lowering=False, debug=False)
    build(nc)
    nc.compile()
    return nc


_CACHE = {}


def _get_nc():
    if "nc" not in _CACHE:
        _CACHE["nc"] = build_nc()
    return _CACHE["nc"]


def kernel(**inputs):
    from concourse import bass_utils
    nc = _get_nc()
    in_maps = prep_inputs(inputs)
    res = bass_utils.run_bass_kernel_spmd(nc, in_maps, core_ids=list(range(8)))
    return assemble_output(res.results)



# revision 18
# speedup vs baseline: 7.9396x; 7.9396x over previous
"""Self-contained Trainium2 (Bass/Tile) kernel for the nn_Encoder problem.

kernel(**inputs) takes the FULL unsharded inputs (as produced by
setup_inputs()) and returns the FULL [4, 2048, 1024] fp32 output.

8-way data-parallel over tokens (2 NeuronCores per batch row, 1024
query-tokens each; K/V computed redundantly per pair => no collectives).

v2: fully SBUF-resident pipeline (no DRAM round-trips for Q/K/V/S/h/G),
fp8 e4m3 DoubleRow matmuls (2 fp8 MACs/cell/cycle) for QKV, scores, P*V,
Wo, and the FFN. Weights are prescaled by 16 (descale folded into existing
fixup ops) to keep fp8 operands in the normal range; the LN1 fixup algebra
keeps the residual backbone in fp32.
"""
import os
import numpy as np

import concourse.bass as bass
import concourse.bacc as bacc
import concourse.mybir as mybir
import concourse.tile as tile

F32 = mybir.dt.float32
F32R = mybir.dt.float32r
BF16 = mybir.dt.bfloat16
F8 = mybir.dt.float8e4
AF = mybir.ActivationFunctionType
ALU = mybir.AluOpType
DR = mybir.MatmulPerfMode.DoubleRow

E = 1024
FF = 4096
B, S = 4, 2048
T = 1024      # own tokens per core
R = 2048      # row tokens (for K/V)
P = 128
NE = E // P   # 8
NT = T // P   # 8
NR = R // P   # 16
EPS = 1e-5
WS = 16.0     # weight prescale (power of 2)


def build(nc):
    # ---- DRAM I/O ----
    xrT = nc.dram_tensor("xrT", [E, R], F32R, kind="ExternalInput")   # row, feature-major
    wq8 = nc.dram_tensor("wq8", [E // 2, 2 * E], F8, kind="ExternalInput")
    wk8 = nc.dram_tensor("wk8", [E // 2, 2 * E], F8, kind="ExternalInput")
    wv8 = nc.dram_tensor("wv8", [E // 2, 2 * E], F8, kind="ExternalInput")
    wo8 = nc.dram_tensor("wo8", [E // 2, 2 * E], F8, kind="ExternalInput")
    w18 = nc.dram_tensor("w18", [E // 2, 2 * FF], F8, kind="ExternalInput")
    w28 = nc.dram_tensor("w28", [FF // 2, 2 * E], F8, kind="ExternalInput")
    xo = nc.dram_tensor("xo", [T, E], F32, kind="ExternalInput")      # own block, token-major
    bq = nc.dram_tensor("bq", [E], F32, kind="ExternalInput")
    bk = nc.dram_tensor("bk", [E], F32, kind="ExternalInput")
    bv = nc.dram_tensor("bv", [E], F32, kind="ExternalInput")
    bo = nc.dram_tensor("bo", [E], F32, kind="ExternalInput")
    b1 = nc.dram_tensor("b1", [FF], F32, kind="ExternalInput")
    b2 = nc.dram_tensor("b2", [E], F32, kind="ExternalInput")
    g3 = nc.dram_tensor("g3", [E], F32, kind="ExternalInput")
    b3 = nc.dram_tensor("b3", [E], F32, kind="ExternalInput")
    ident_in = nc.dram_tensor("ident_in", [P, P], BF16, kind="ExternalInput")
    ones_in = nc.dram_tensor("ones_in", [P, 1], F32R, kind="ExternalInput")
    ones8_in = nc.dram_tensor("ones8_in", [P, 32], F8, kind="ExternalInput")
    y = nc.dram_tensor("y", [T, E], F32, kind="ExternalOutput")

    def bcast_ap(vec_t, n):
        a = vec_t.ap()
        return bass.AP(tensor=a.tensor, offset=a.offset, ap=[[0, P], [1, n]])

    with tile.TileContext(nc) as tc:
        consts_cm = tc.tile_pool(name="consts", bufs=1)
        consts = consts_cm.__enter__()
        dram_cm = tc.tile_pool(name="dram", bufs=1, space="DRAM")
        dram = dram_cm.__enter__()

        ident_sb = consts.tile([P, P], BF16, tag="ident")
        nc.sync.dma_start(out=ident_sb, in_=ident_in.ap())
        ones_sb = consts.tile([P, 1], F32R, tag="ones")
        nc.sync.dma_start(out=ones_sb, in_=ones_in.ap())
        ones8_sb = consts.tile([P, 2, 16], F8, tag="ones8")
        nc.sync.dma_start(out=ones8_sb, in_=ones8_in.ap())
        eps_row = consts.tile([1, 1], F32, tag="eps_row")
        nc.vector.memset(eps_row, EPS)
        eps_col = consts.tile([P, 1], F32, tag="eps_col")
        nc.vector.memset(eps_col, EPS)
        bq_sb = consts.tile([P, NE], F32, tag="bq")
        nc.sync.dma_start(out=bq_sb, in_=bq.ap().rearrange("(t p) -> p t", p=P))
        bk_sb = consts.tile([P, NE], F32, tag="bk")
        nc.sync.dma_start(out=bk_sb, in_=bk.ap().rearrange("(t p) -> p t", p=P))
        b1_sb = consts.tile([P, FF // P], F32, tag="b1")
        nc.sync.dma_start(out=b1_sb, in_=b1.ap().rearrange("(t p) -> p t", p=P))
        bv_b = consts.tile([P, E], F32, tag="bv_b")
        nc.sync.dma_start(out=bv_b, in_=bcast_ap(bv, E))
        bo_b = consts.tile([P, E], F32, tag="bo_b")
        nc.sync.dma_start(out=bo_b, in_=bcast_ap(bo, E))
        b2_b = consts.tile([P, E], F32, tag="b2_b")
        nc.sync.dma_start(out=b2_b, in_=bcast_ap(b2, E))
        g3_b = consts.tile([P, E], F32, tag="g3_b")
        nc.sync.dma_start(out=g3_b, in_=bcast_ap(g3, E))
        b3_b = consts.tile([P, E], F32, tag="b3_b")
        nc.sync.dma_start(out=b3_b, in_=bcast_ap(b3, E))

        stat_d = dram.tile([4, R], F32, tag="stat_d")   # mean, rstd rows + recip row

        def encoder():
            with tc.tile_pool(name="keep", bufs=1) as keepp:
                recip_col = keepp.tile([P, NT], F32, tag="recip_col")
                vppx_cm = tc.tile_pool(name="vppx", bufs=1)
                vppx = vppx_cm.__enter__()
                vp = [vppx.tile([P, 2, E], F8, tag=f"vp{j}", name=f"vp{j}") for j in range(8)]
                expp = [vppx.tile([P, 2, T], F8, tag=f"ex{j}", name=f"ex{j}")
                        for j in range(8)]
                qkp_cm = tc.tile_pool(name="qkp", bufs=1)
                qkp = qkp_cm.__enter__()
                qp = [qkp.tile([P, 2, T], F8, tag=f"qp{j}", name=f"qp{j}") for j in range(4)]
                kp = [qkp.tile([P, 2, R], F8, tag=f"kp{j}", name=f"kp{j}") for j in range(4)]

                # ============ stage A: x load, LN1 stats, QKV (fp8 DR) ============
                with tc.tile_pool(name="xw", bufs=1) as xw, \
                     tc.tile_pool(name="wqkv", bufs=1) as wqkv, \
                     tc.tile_pool(name="bcp", bufs=1) as bcp:
                    xn8 = [xw.tile([P, 2, R], F8, tag=f"xn8{j}", name=f"xn8{j}")
                           for j in range(4)]
                    wq_j, wk_j, wv_j = [], [], []
                    for lst, dt_ in ((wq_j, wq8), (wk_j, wk8), (wv_j, wv8)):
                        for j in range(4):
                            t = wqkv.tile([P, 2, E], F8, tag="w", bufs=8,
                                          name=f"w_{len(lst)}_{j}")
                            nc.sync.dma_start(out=t, in_=dt_.ap()[j * P:(j + 1) * P, :])
                            lst.append(t)
                    r_b = bcp.tile([P, R], F32, tag="r_b")
                    m_b = bcp.tile([P, R], F32, tag="m_b")
                    with tc.tile_pool(name="sAsq", bufs=3) as sqp, \
                         tc.tile_pool(name="sAx32", bufs=2) as x32p, \
                         tc.tile_pool(name="sArow", bufs=1) as rowp, \
                         tc.tile_pool(name="sArps", bufs=1, space="PSUM") as rpsA:
                        ps_s = [rpsA.tile([1, 512], F32, tag=f"ps_s{c}", name=f"ps_s{c}")
                                for c in range(4)]
                        ps_q = [rpsA.tile([1, 512], F32, tag=f"ps_q{c}", name=f"ps_q{c}")
                                for c in range(4)]
                        for k in range(NE):
                            for c in range(4):
                                cs = slice(c * 512, (c + 1) * 512)
                                x_kc = x32p.tile([P, 512], F32R, tag="x32", bufs=3,
                                                 name=f"x{k}_{c}")
                                nc.sync.dma_start(out=x_kc,
                                                  in_=xrT.ap()[k * P:(k + 1) * P, cs])
                                sq = sqp.tile([P, 512], F32R, tag="sq", bufs=2)
                                nc.vector.tensor_mul(sq, x_kc[:], x_kc[:])
                                nc.tensor.matmul(ps_s[c][:], ones_sb[:], x_kc[:],
                                                 start=(k == 0), stop=(k == NE - 1))
                                nc.tensor.matmul(ps_q[c][:], ones_sb[:], sq[:],
                                                 start=(k == 0), stop=(k == NE - 1))
                        qrow = rowp.tile([1, R], F32, tag="qrow")
                        mean = rowp.tile([1, R], F32, tag="mean")
                        for c in range(4):
                            cs = slice(c * 512, (c + 1) * 512)
                            nc.vector.tensor_scalar_mul(mean[:, cs], ps_s[c][:], 1.0 / E)
                            nc.vector.tensor_scalar_mul(qrow[:, cs], ps_q[c][:], 1.0 / E)
                        msq = rowp.tile([1, R], F32, tag="msq")
                        nc.vector.tensor_mul(msq, mean[:], mean[:])
                        var = rowp.tile([1, R], F32, tag="var")
                        nc.vector.tensor_tensor(out=var, in0=qrow[:], in1=msq[:],
                                                op=ALU.subtract)
                        sd = rowp.tile([1, R], F32, tag="msq")
                        nc.scalar.activation(out=sd, in_=var[:], func=AF.Sqrt,
                                             bias=eps_row[:], scale=1.0)
                        rstd = rowp.tile([1, R], F32, tag="qrow")
                        nc.vector.reciprocal(rstd, sd[:])
                        nc.gpsimd.partition_broadcast(r_b, rstd[:])
                        nc.gpsimd.partition_broadcast(m_b, mean[:])
                        for c in range(4):
                            for k in range(NE):
                                cs = slice(c * 512, (c + 1) * 512)
                                x_kc = x32p.tile([P, 512], F32R, tag="x32b", bufs=4,
                                                 name=f"xr{k}_{c}")
                                nc.sync.dma_start(out=x_kc,
                                                  in_=xrT.ap()[k * P:(k + 1) * P, cs])
                                xm = sqp.tile([P, 512], F32R, tag="sq", bufs=2, name="xm")
                                nc.vector.tensor_tensor(out=xm, in0=x_kc[:],
                                                        in1=m_b[:, cs], op=ALU.subtract)
                                nc.gpsimd.tensor_mul(xn8[k // 2][:, k % 2, cs], xm[:],
                                                      r_b[:, cs])

                    with tc.tile_pool(name="sAfix", bufs=2) as fxp, \
                         tc.tile_pool(name="sAps", bufs=4, space="PSUM") as psA:
                        for m in range(NE):
                            for qc in range(2):
                                qs = slice(qc * 512, (qc + 1) * 512)
                                ps = psA.tile([P, 512], F32, tag="psqkv")
                                for j in range(4):
                                    nc.tensor.matmul(ps[:], wq_j[j][:, :, m * P:(m + 1) * P],
                                                     xn8[j][:, :, qs], perf_mode=DR,
                                                     start=(j == 0), stop=(j == 3))
                                nc.scalar.activation(out=qp[m // 2][:, m % 2, qs],
                                                     in_=ps[:], func=AF.Identity,
                                                     bias=bq_sb[:, m:m + 1], scale=1.0 / WS)
                        for m in range(NE):
                            for kc in range(4):
                                ks = slice(kc * 512, (kc + 1) * 512)
                                ps = psA.tile([P, 512], F32, tag="psqkv")
                                for j in range(4):
                                    nc.tensor.matmul(ps[:], wk_j[j][:, :, m * P:(m + 1) * P],
                                                     xn8[j][:, :, ks], perf_mode=DR,
                                                     start=(j == 0), stop=(j == 3))
                                nc.scalar.activation(out=kp[m // 2][:, m % 2, ks],
                                                     in_=ps[:], func=AF.Identity,
                                                     bias=bk_sb[:, m:m + 1], scale=1.0 / WS)
                        for rm in range(NR):
                            for c in range(2):
                                cs = slice(c * 512, (c + 1) * 512)
                                ps = psA.tile([P, 512], F32, tag="psqkv")
                                for j in range(4):
                                    nc.tensor.matmul(ps[:],
                                                     xn8[j][:, :, rm * P:(rm + 1) * P],
                                                     wv_j[j][:, :, cs], perf_mode=DR,
                                                     start=(j == 0), stop=(j == 3))
                                nc.vector.scalar_tensor_tensor(
                                    out=vp[rm // 2][:, rm % 2, cs], in0=ps[:],
                                    scalar=1.0 / WS, in1=bv_b[:, cs],
                                    op0=ALU.mult, op1=ALU.add)

                # ============ stage B: S^T = K^T Q, exp, sums ============
                with tc.tile_pool(name="sBsm", bufs=2) as smp, \
                     tc.tile_pool(name="sBps", bufs=4, space="PSUM") as psB, \
                     tc.tile_pool(name="sBsum", bufs=2, space="PSUM") as psSum:
                    recip_row = smp.tile([1, T], F32, tag="recip_row", bufs=1)
                    for qc in range(2):
                        qs = slice(qc * 512, (qc + 1) * 512)
                        ps_sum = psSum.tile([1, 512], F32, tag="ps_sum")
                        for kt in range(NR):
                            ps = psB.tile([P, 512], F32, tag="pss")
                            for j in range(4):
                                nc.tensor.matmul(ps[:], kp[j][:, :, kt * P:(kt + 1) * P],
                                                 qp[j][:, :, qs], perf_mode=DR,
                                                 start=(j == 0), stop=(j == 3))
                            nc.scalar.activation(out=expp[kt // 2][:, kt % 2, qs], in_=ps[:],
                                                 func=AF.Exp, scale=1.0 / 32.0)
                            if kt % 2 == 1:
                                jj = kt // 2
                                nc.tensor.matmul(ps_sum[:], ones8_sb[:, :, 0:1],
                                                 expp[jj][:, :, qs], perf_mode=DR,
                                                 start=(jj == 0), stop=(jj == 7))
                        rsum = smp.tile([1, 512], F32, tag="rsum")
                        nc.vector.tensor_copy(out=rsum, in_=ps_sum[:])
                        rcp = smp.tile([1, 512], F32, tag="rcp")
                        nc.vector.reciprocal(rcp, rsum[:])
                        # fold: /WS for Wo weights, *8 for AO/8 fp8 copy
                        nc.vector.tensor_scalar_mul(recip_row[:, qs], rcp[:], 8.0 / WS)
                    nc.sync.dma_start(out=stat_d[2:3, 0:T], in_=recip_row[:])
                nc.sync.dma_start(out=recip_col,
                                  in_=stat_d[2:3, 0:T].rearrange("a (t p) -> (a p) t", p=P))
                qkp_cm.__exit__(None, None, None)

                # ============ stage C: AO = V^T expS^T, O = AO^T Wo, h ============
                h_t = [keepp.tile([P, E], F32, tag=f"h{t}", name=f"h{t}")
                       for t in range(NT)]
                with tc.tile_pool(name="aop_p", bufs=1) as aop_p, \
                     tc.tile_pool(name="wop", bufs=1) as wop, \
                     tc.tile_pool(name="sCw", bufs=2) as wkc, \
                     tc.tile_pool(name="sCps", bufs=3, space="PSUM") as psC:
                    aop = [aop_p.tile([P, 2, T], F8, tag=f"ao{j}", name=f"ao{j}")
                           for j in range(4)]
                    wo_j = []
                    for j in range(4):
                        t = wop.tile([P, 2, E], F8, tag=f"wo_{j}", name=f"wo_{j}")
                        nc.sync.dma_start(out=t, in_=wo8.ap()[j * P:(j + 1) * P, :])
                        wo_j.append(t)
                    for m in range(NE):
                        for qc in range(2):
                            qs = slice(qc * 512, (qc + 1) * 512)
                            psa = psC.tile([P, 512], F32, tag="psa")
                            for j in range(8):
                                nc.tensor.matmul(psa[:], vp[j][:, :, m * P:(m + 1) * P],
                                                 expp[j][:, :, qs], perf_mode=DR,
                                                 start=(j == 0), stop=(j == 7))
                            # AO/8 into fp8 (range safety); folded back via recip
                            nc.scalar.activation(out=aop[m // 2][:, m % 2, qs], in_=psa[:],
                                                 func=AF.Copy, scale=0.125)
                    for tm in range(NT):
                        pso = psC.tile([P, E], F32, tag="pso", bufs=2)
                        for c in range(2):
                            cs = slice(c * 512, (c + 1) * 512)
                            for j in range(4):
                                nc.tensor.matmul(pso[:, cs],
                                                 aop[j][:, :, tm * P:(tm + 1) * P],
                                                 wo_j[j][:, :, cs], perf_mode=DR,
                                                 start=(j == 0), stop=(j == 3))
                        xo_t = wkc.tile([P, E], F32, tag="xo_t")
                        nc.sync.dma_start(out=xo_t, in_=xo.ap()[tm * P:(tm + 1) * P, :])
                        xob = wkc.tile([P, E], F32, tag="xob")
                        nc.vector.tensor_add(xob, xo_t[:], bo_b[:])
                        t0 = wkc.tile([P, E], F32, tag="t0")
                        nc.vector.tensor_scalar_mul(t0, pso[:], recip_col[:, tm:tm + 1])
                        nc.gpsimd.tensor_add(h_t[tm], t0[:], xob[:])
                vppx_cm.__exit__(None, None, None)

                # ============ stage D: LN2, transpose, F1/F2 (fp8 DR), LN3 ============
                with tc.tile_pool(name="sDhn", bufs=1) as hnp, \
                     tc.tile_pool(name="sDg", bufs=1) as gp_p, \
                     tc.tile_pool(name="sDt", bufs=3) as t6, \
                     tc.tile_pool(name="sDst", bufs=2) as st6:
                    hp = [hnp.tile([P, 2, T], F8, tag=f"hp{j}", name=f"hp{j}")
                          for j in range(4)]
                    gp = [gp_p.tile([P, 2, T], F8, tag=f"gp{j}", name=f"gp{j}")
                          for j in range(16)]
                    w1_j = []
                    for j in range(4):
                        t = hnp.tile([P, 2, FF], F8, tag=f"w1_{j}", name=f"w1_{j}")
                        nc.sync.dma_start(out=t, in_=w18.ap()[j * P:(j + 1) * P, :])
                        w1_j.append(t)
                    with tc.tile_pool(name="sDtp", bufs=2, space="PSUM") as psDt:
                        for tm in range(NT):
                            stats = st6.tile([P, 2, 6], F32, tag="stats")
                            hg = h_t[tm][:].rearrange("p (g d) -> p g d", g=2)
                            for g in range(2):
                                nc.vector.bn_stats(out=stats[:, g, :], in_=hg[:, g, :])
                            mv = st6.tile([P, 2], F32, tag="mv")
                            nc.vector.bn_aggr(out=mv, in_=stats[:])
                            sd = st6.tile([P, 1], F32, tag="sd")
                            nc.scalar.activation(out=sd, in_=mv[:, 1:2], func=AF.Sqrt,
                                                 bias=eps_col[:], scale=1.0)
                            rinv = st6.tile([P, 1], F32, tag="rinv")
                            nc.vector.reciprocal(rinv, sd[:])
                            hn = t6.tile([P, E], BF16, tag="hn", bufs=2, name="hn")
                            nc.vector.tensor_scalar(out=hn, in0=h_t[tm][:],
                                                    scalar1=mv[:, 0:1],
                                                    scalar2=rinv[:], op0=ALU.subtract,
                                                    op1=ALU.mult)
                            for et in range(NE):
                                tp = psDt.tile([P, P], BF16, tag="tp5")
                                nc.tensor.transpose(tp, hn[:, et * P:(et + 1) * P],
                                                    ident_sb[:])
                                nc.scalar.copy(
                                    out=hp[et // 2][:, et % 2, tm * P:(tm + 1) * P],
                                    in_=tp[:])
                    with tc.tile_pool(name="w2p", bufs=1) as w2p:
                        w2_j = []
                        for j in range(16):
                            t = w2p.tile([P, 2, E], F8, tag=f"w2_{j}", name=f"w2_{j}")
                            nc.sync.dma_start(out=t, in_=w28.ap()[j * P:(j + 1) * P, :])
                            w2_j.append(t)
                        with tc.tile_pool(name="sDpsgf", bufs=1, space="PSUM") as psgf:
                            for f in range(FF // P):
                                for qc in range(2):
                                    qs = slice(qc * 512, (qc + 1) * 512)
                                    psg = psgf.tile([P, 512], F32, tag="psg", bufs=2)
                                    for j in range(4):
                                        nc.tensor.matmul(psg[:],
                                                         w1_j[j][:, :, f * P:(f + 1) * P],
                                                         hp[j][:, :, qs], perf_mode=DR,
                                                         start=(j == 0), stop=(j == 3))
                                    nc.scalar.activation(out=gp[f // 2][:, f % 2, qs],
                                                         in_=psg[:], func=AF.Relu,
                                                         bias=b1_sb[:, f:f + 1],
                                                         scale=1.0 / WS)
                            for tm in range(NT):
                                psf = psgf.tile([P, E], F32, tag="psf", bufs=3)
                                for j in range(16):
                                    for c in range(2):
                                        cs = slice(c * 512, (c + 1) * 512)
                                        nc.tensor.matmul(psf[:, cs],
                                                         gp[j][:, :, tm * P:(tm + 1) * P],
                                                         w2_j[j][:, :, cs], perf_mode=DR,
                                                         start=(j == 0), stop=(j == 15))
                                t1 = t6.tile([P, E], F32, tag="chain", name="t1")
                                nc.vector.scalar_tensor_tensor(out=t1, in0=psf[:],
                                                               scalar=1.0 / WS,
                                                               in1=h_t[tm][:],
                                                               op0=ALU.mult, op1=ALU.add)
                                op = t6.tile([P, E], F32, tag="chain", name="op")
                                nc.vector.tensor_add(op, t1[:], b2_b[:])
                                stats = st6.tile([P, 2, 6], F32, tag="stats7")
                                og = op[:].rearrange("p (g d) -> p g d", g=2)
                                for g in range(2):
                                    nc.vector.bn_stats(out=stats[:, g, :], in_=og[:, g, :])
                                mv = st6.tile([P, 2], F32, tag="mv7")
                                nc.vector.bn_aggr(out=mv, in_=stats[:])
                                sd = st6.tile([P, 1], F32, tag="sd7")
                                nc.scalar.activation(out=sd, in_=mv[:, 1:2], func=AF.Sqrt,
                                                     bias=eps_col[:], scale=1.0)
                                rinv = st6.tile([P, 1], F32, tag="rinv7")
                                nc.vector.reciprocal(rinv, sd[:])
                                n = t6.tile([P, E], F32, tag="chain", name="n")
                                nc.vector.tensor_scalar(out=n, in0=op[:], scalar1=mv[:, 0:1],
                                                        scalar2=rinv[:], op0=ALU.subtract,
                                                        op1=ALU.mult)
                                yg = t6.tile([P, E], F32, tag="chain", name="yg")
                                nc.vector.tensor_mul(yg, n[:], g3_b[:])
                                yt = t6.tile([P, E], F32, tag="chain", name="yt")
                                nc.vector.tensor_add(yt, yg[:], b3_b[:])
                                nc.sync.dma_start(out=y.ap()[tm * P:(tm + 1) * P, :],
                                                  in_=yt[:])

        for _rep in range(int(os.environ.get("ENC_REPS", "1"))):
            encoder()

        consts_cm.__exit__(None, None, None)
        dram_cm.__exit__(None, None, None)


# ======================= host-side prep / assembly =========================

def prep_inputs(inputs):
    import ml_dtypes
    F8NP = ml_dtypes.float8_e4m3
    src = np.asarray(inputs["src_embs"], np.float32)   # [B, S, E]
    g1 = np.asarray(inputs["g1"], np.float32)
    b1ln = np.asarray(inputs["b1"], np.float32)
    g2 = np.asarray(inputs["g2"], np.float32)
    b2ln = np.asarray(inputs["b2"], np.float32)

    Wq, bq = np.asarray(inputs["Wq_w"], np.float32), np.asarray(inputs["Wq_b"], np.float32)
    Wk, bk = np.asarray(inputs["Wk_w"], np.float32), np.asarray(inputs["Wk_b"], np.float32)
    Wv, bv = np.asarray(inputs["Wv_w"], np.float32), np.asarray(inputs["Wv_b"], np.float32)
    Wo, bo = np.asarray(inputs["Wo_w"], np.float32), np.asarray(inputs["Wo_b"], np.float32)
    W1, b1f = np.asarray(inputs["W1_w"], np.float32), np.asarray(inputs["W1_b"], np.float32)
    W2, b2f = np.asarray(inputs["W2_w"], np.float32), np.asarray(inputs["W2_b"], np.float32)

    def pairize(WT):
        # WT [E_in, M] fp32 -> quantized fp8 pair layout [E_in//2, 2*M]
        # row r = j*128+p, col = i*M+m  with e = 256j + 128i + p
        Ein, M = WT.shape
        W8 = (WT * WS).astype(F8NP)
        deq = W8.astype(np.float32) / WS
        arr = W8.reshape(Ein // 256, 2, P, M).transpose(0, 2, 1, 3).reshape(Ein // 2, 2 * M)
        return np.ascontiguousarray(arr), deq

    wq8, _ = pairize((Wq * g1[None, :]).T)
    wk8, _ = pairize((Wk * g1[None, :]).T)
    wv8, _ = pairize((Wv * g1[None, :]).T)
    wo8, _ = pairize(Wo.T)
    w18, _ = pairize((W1 * g2[None, :]).T)
    w28, _ = pairize(W2.T)

    bq_eff = (bq + Wq @ b1ln).astype(np.float32)
    bk_eff = (bk + Wk @ b1ln).astype(np.float32)
    bv_eff = (bv + Wv @ b1ln).astype(np.float32)
    b1_eff = (b1f + W1 @ b2ln).astype(np.float32)

    shared = dict(
        wq8=wq8, wk8=wk8, wv8=wv8, wo8=wo8, w18=w18, w28=w28,
        bq=bq_eff, bk=bk_eff, bv=bv_eff, bo=bo,
        b1=b1_eff, b2=b2f,
        g3=np.asarray(inputs["g3"], np.float32), b3=np.asarray(inputs["b3"], np.float32),
        ident_in=np.eye(P, dtype=ml_dtypes.bfloat16),
        ones_in=np.ones((P, 1), np.float32),
        ones8_in=np.ones((P, 32), F8NP),
    )
    in_maps = []
    for c in range(8):
        b, half = c // 2, c % 2
        row = src[b]
        own = row[half * T:(half + 1) * T]
        other = row[(1 - half) * T:(2 - half) * T]
        xr = np.concatenate([own, other], axis=0)
        m = dict(shared)
        m["xrT"] = np.ascontiguousarray(xr.T)
        m["xo"] = np.ascontiguousarray(own)
        in_maps.append(m)
    return in_maps


def assemble_output(results):
    out = np.zeros((B, S, E), np.float32)
    for c in range(8):
        b, half = c // 2, c % 2
        out[b, half * T:(half + 1) * T] = results[c]["y"]
    return out


def build_nc():
    nc = bacc.Bacc("TRN2", target_bir_lowering=False, debug=False)
    build(nc)
    nc.compile()
    return nc


_CACHE = {}


def _get_nc():
    if "nc" not in _CACHE:
        _CACHE["nc"] = build_nc()
    return _CACHE["nc"]


def kernel(**inputs):
    from concourse import bass_utils
    nc = _get_nc()
    in_maps = prep_inputs(inputs)
    res = bass_utils.run_bass_kernel_spmd(nc, in_maps, core_ids=list(range(8)))
    return assemble_output(res.results)
